# revision 23
# baseline (speedup 1.0000x reference)
"""Back-projection (nn_BackProjectionNet) Trainium2 Bass kernel.

Full inputs in, full outputs out. Sharding: z (last dim, 192) split over 8
cores, 24 z-planes each; no collectives (each core fully independent).

Math: out[y,x,z] = sum_n lerp_x(padded_slice_n; sx_n(y,x)) / (120 + 1e-11)
with sx = sp*yc + cp*xc + C. The 120 angles fold 8-ways onto 16 base angles
in [0,45deg] ("sin convention"), 31 (base, xmirror) sets.

Per set, with K[x] = floor(cpx*xc)-KMIN, S[y] = floor(spa*yc+C)+KMIN,
k = frac_x, r = frac_y, wB = max(k+r-1, 0), G_j = C[S[y]+K[x]+j]:

  out_set = G0 + k(G1-G0) + r(G1-G0) + wB(G0-2G1+G2)        (exact)

Device pipeline per (base, y-quarter):
  1. affine-segment shear DMA (one 4D instruction per maximal slope-1
     segment of S - ~5/quarter vs ~26 constant-S runs):
       T[u<uspan, pair*1536 + y*48 + d48] = C[S[y]+u, b, pair-slots, z]
     pair-major blocks so every matmul rhs chunk is CONTIGUOUS - the PE
     runs strided rhs ~4x slower (~970ns vs 233ns per 512-col matmul).
  2. PE streams into PSUM acc[x, y*48+d48] (both pairs accumulate into the
     same columns - the A/B member sums the unwind needs anyway):
       S1: L0 @ T      L0 = (1-k) at row K, k at K+1   -> G0 + k(G1-G0)
       S2: D1 @ T_r    D1 = -+1 at K,K+1; T_r = T*r[y] (DVE bf16)
       S3: D2 @ T -> pd (transient PSUM); D2 = +1,-2,+1 at K..K+2
  3. Act: pc = bf16(pd) (Pool can't read PSUM); DVE: m = wB .* pc at
     2x bf16 rate; Pool: aw += m
  4. evac: OUT-quarter = acc + cvt(aw), flat (acc layout == out_t layout)
Unwind: frame B is PE-transposed and added, scale by 1/(120+1e-11).
Host does only layout (pad, z-shard, slice flips, final transpose).
"""

import math
import numpy as np

import concourse.bacc as bacc
import concourse.mybir as mybir
from concourse import tile
from concourse.ap import AP
from concourse.bass_utils import run_bass_kernel_spmd

NA, LR, LZ, PAD = 120, 128, 192, 27
LP = LR + 2 * PAD          # 182
CEN = (LP - 1) / 2.0       # 90.5
N_CORES = 8
ZC = LZ // N_CORES         # 24
CROWS = 288                # padded combined-slice rows
NBASE = 16
NSLOT = 4                  # job slots per base in C buffer
DT = mybir.dt
INV_NORM = float(np.float32(1.0 / (120.0 + 1e-11)))

F32, BF16 = DT.float32, DT.bfloat16
CJ = NBASE * NSLOT * ZC    # C row stride in elements = 1536


# ---------------------------------------------------------------- host math

def _job_slots():
    """Per base: slot -> (plain_member, flipped_member) or None.
    Slot order: [plain-A, plain-B, xm-A, xm-B]."""
    slots = {}
    for b in range(NBASE):
        if b == 0:
            slots[b] = [(60, 0), (90, 30), None, None]
        elif b == 15:
            slots[b] = [(75, 15), None, None, (105, 45)]
        else:
            slots[b] = [((60 + b) % 120, b), (90 - b, 30 - b),
                        (60 - b, 120 - b), (90 + b, 30 + b)]
    return slots


def _base_tables(b, xm):
    a = 2 * math.pi * b / NA
    cpa, spa = math.sin(a), math.cos(a)
    cpx = -cpa if xm else cpa
    yc = np.arange(PAD, PAD + LR, dtype=np.float64) - CEN
    xc = np.arange(PAD, PAD + LR, dtype=np.float64) - CEN
    ay = spa * yc + CEN
    bx = cpx * xc
    Sf, Kf = np.floor(ay), np.floor(bx)
    KMIN = int(Kf.min())
    K = (Kf - KMIN).astype(np.int64)        # [x] >= 0
    S = (Sf + KMIN).astype(np.int64)        # [y]
    r = (ay - Sf).astype(np.float64)        # frac_y [y]
    k = (bx - Kf).astype(np.float64)        # frac_x [x]
    wB = np.maximum(r[None, :] + k[:, None] - 1.0, 0.0)   # [x, y]
    return S, K, r, k, wB


def host_prep():
    """Build all constant tables + plans."""
    slots = _job_slots()
    sets = []            # (b, xm)
    for b in range(NBASE):
        sets.append((b, False))
        if b != 0:
            sets.append((b, True))
    nset = len(sets)     # 31
    L = np.zeros((nset, 3, 128, 128), np.float32)   # [set, stream, u, x]
    W2 = np.zeros((nset, 128, 128), np.float32)     # [set, x, y] = wB
    Rtab = np.zeros((NBASE, 128), np.float32)       # [base, y] = r
    skip3 = {}
    Sbase = {}
    xs = np.arange(128)
    for i, (b, xm) in enumerate(sets):
        S, K, r, k, wB = _base_tables(b, xm)
        if b in Sbase:
            assert np.array_equal(Sbase[b][0], S)
            assert np.allclose(Sbase[b][1], r)
        Sbase[b] = (S, r)
        Rtab[b] = r.astype(np.float32)
        L[i, 0][K, xs] = 1.0 - k
        L[i, 0][K + 1, xs] = k
        L[i, 1][K, xs] = -1.0
        L[i, 1][K + 1, xs] = 1.0
        L[i, 2][K, xs] = 1.0
        L[i, 2][K + 1, xs] = -2.0
        L[i, 2][K + 2, xs] = 1.0
        W2[i] = wB
        skip3[i] = bool(np.all(wB == 0.0))
    rzero = {b: bool(np.all(Rtab[b] == 0.0)) for b in range(NBASE)}
    # per-base tap-row span (rows with any nonzero L entry, over live streams)
    uspan = {}
    segs = {}
    for b in range(NBASE):
        hi = 0
        for i, (bb, xm) in enumerate(sets):
            if bb != b:
                continue
            streams = [0] + ([] if rzero[b] else [1]) \
                + ([] if skip3[i] else [2])
            for j in streams:
                rows = np.nonzero(np.abs(L[i, j]).max(axis=1) > 0)[0]
                hi = max(hi, int(rows.max()) + 1)
        uspan[b] = hi
        # affine (slope-1) segments of S within each y-quarter
        S = Sbase[b][0]
        for q in range(4):
            rr = []
            y = 32 * q
            while y < 32 * (q + 1):
                y1 = y
                while y1 + 1 < 32 * (q + 1) and S[y1 + 1] == S[y1] + 1:
                    y1 += 1
                rr.append((y, y1 - y + 1, int(S[y])))
                assert S[y] >= 0 and S[y] + (y1 - y) + hi <= CROWS, (b, q, y)
                y = y1 + 1
            segs[(b, q)] = rr
    ident = np.eye(128, dtype=np.float32)
    return dict(slots=slots, sets=sets, L=L, W2=W2, Rtab=Rtab, ident=ident,
                segs=segs, uspan=uspan, skip3=skip3, rzero=rzero)


def host_inputs(image, core):
    """Per-core input arrays. image [1,120,128,192] f32."""
    z0 = core * ZC
    img = np.asarray(image)[0, :, :, z0:z0 + ZC]               # [120,128,ZC]
    img_p = np.pad(img, ((0, 0), (PAD, PAD), (0, 0)))          # [120,182,ZC]
    slots = _job_slots()
    sp = np.zeros((NBASE * NSLOT, LP, ZC), np.float32)
    sf = np.zeros((NBASE * NSLOT, LP, ZC), np.float32)
    for b in range(NBASE):
        for s in range(NSLOT):
            j = slots[b][s]
            if j is None:
                continue
            mp, mf = j
            sp[b * NSLOT + s] = img_p[mp]
            sf[b * NSLOT + s] = img_p[mf][::-1]
    return {"slices_p": sp, "slices_f": sf}


# ---------------------------------------------------------------- device

def build_nc(tabs, repeat=1, nbases=NBASE, nquarters=4):
    sets, segs, skip3, rzero = tabs["sets"], tabs["segs"], tabs["skip3"], \
        tabs["rzero"]
    uspan = tabs["uspan"]
    nset = len(sets)
    set_idx = {bs: i for i, bs in enumerate(sets)}

    nc = bacc.Bacc("TRN2", target_bir_lowering=False, debug=False,
                   num_devices=N_CORES)
    d_sp = nc.dram_tensor("slices_p", [NBASE * NSLOT, LP, ZC], F32,
                          kind="ExternalInput")
    d_sf = nc.dram_tensor("slices_f", [NBASE * NSLOT, LP, ZC], F32,
                          kind="ExternalInput")
    d_L = nc.dram_tensor("l_tab", [nset * 3, 128, 128], BF16,
                         kind="ExternalInput")
    d_W = nc.dram_tensor("w_tab", [nset, 128, 128], BF16,
                         kind="ExternalInput")
    d_R = nc.dram_tensor("r_tab", [128, NBASE * 128], BF16,
                         kind="ExternalInput")
    d_I = nc.dram_tensor("ident", [128, 128], F32, kind="ExternalInput")
    d_out = nc.dram_tensor("out", [128, 128, ZC], F32, kind="ExternalOutput")

    with tile.TileContext(nc) as tc:
        with tc.tile_pool(name="const", bufs=1) as cpool, \
             tc.tile_pool(name="work", bufs=3) as wpool, \
             tc.tile_pool(name="once", bufs=1) as opool, \
             tc.tile_pool(name="accs", bufs=1) as apool, \
             tc.tile_pool(name="dram", bufs=1, space="DRAM") as dpool, \
             tc.tile_pool(name="psum", bufs=1, space="PSUM") as ppool:

            d_C = dpool.tile([CROWS * NBASE * NSLOT * ZC], BF16, tag="cbuf")
            c_base = d_C[:].tensor

            # ---- constants to SBUF (outside timing loop) ----
            t_L = cpool.tile([128, nset * 3 * 128], BF16, tag="ltab")
            nc.sync.dma_start(
                out=t_L[:],
                in_=AP(d_L[:].tensor, 0,
                       [[128, 128], [128 * 128, nset * 3], [1, 128]]))
            t_W = cpool.tile([128, nset * 128], BF16, tag="wtab")
            nc.sync.dma_start(
                out=t_W[:],
                in_=AP(d_W[:].tensor, 0,
                       [[128, 128], [128 * 128, nset], [1, 128]]))
            t_R = cpool.tile([128, NBASE * 128], BF16, tag="rtab")
            nc.sync.dma_start(out=t_R[:], in_=d_R[:])
            t_I = cpool.tile([128, 128], F32, tag="ident")
            nc.sync.dma_start(out=t_I[:], in_=d_I[:])

            def L_ap(si, j, us):   # lhsT [us, 128] bf16
                return AP(t_L[:].tensor, (si * 3 + j) * 128,
                          [[nset * 3 * 128, us], [1, 128]])

            def W_ap(si, q):       # [128, (y32), (d48 bcast)] bf16
                return AP(t_W[:].tensor, si * 128 + 32 * q,
                          [[nset * 128, 128], [1, 32], [0, 48]])

            dma_eng = [nc.sync, nc.scalar]

            # ---- zero C buffer once (combine only writes rows < 182) ----
            t_z = opool.tile([128, 3456], BF16, tag="zero")
            nc.gpsimd.memset(t_z[:], 0)
            nc.sync.dma_start(
                out=AP(c_base, 0, [[3456, 128], [1, 3456]]),
                in_=t_z[:])

            def body():
                # ---- combine slices: C = P + flip(F) (host pre-flipped) ----
                t_p = opool.tile([64, LP * ZC], F32, tag="slp")
                t_f = opool.tile([64, LP * ZC], F32, tag="slf")
                nc.sync.dma_start(
                    out=t_p[:],
                    in_=AP(d_sp[:].tensor, 0, [[LP * ZC, 64], [1, LP * ZC]]))
                nc.scalar.dma_start(
                    out=t_f[:],
                    in_=AP(d_sf[:].tensor, 0, [[LP * ZC, 64], [1, LP * ZC]]))
                t_c = opool.tile([64, LP * ZC], BF16, tag="slc")
                nc.vector.tensor_add(t_c[:], t_p[:], t_f[:])
                nc.sync.dma_start(
                    out=AP(c_base, 0, [[ZC, 64], [CJ, LP], [1, ZC]]),
                    in_=t_c[:])

                # ---- main loop ----
                out_t = apool.tile([128, 128 * 48], F32, tag="outbuf")
                ndma = [0]
                ndeint = [0]
                deint_eng = [nc.sync, nc.scalar]
                for q in range(nquarters):
                    acc = ppool.tile([128, 1536], F32, tag="acc")
                    aw = apool.tile([128, 1536], BF16, tag="aw")
                    nc.gpsimd.memset(aw[:], 0)
                    # per-ch acc matmul counts for start/stop flags
                    n_accmm = 0
                    for b in range(nbases):
                        n_accmm += (2 if b != 0 else 1)     # plain,xm
                        if not rzero[b]:
                            n_accmm += (2 if b != 0 else 1)
                    mm_done = [0] * 3
                    first_acc = [True] * 3
                    for b in range(nbases):
                        us = uspan[b]
                        # affine-segment shear DMA -> Traw[u<us, y*96 + d96]
                        t_Tw = wpool.tile([128, 96 * 32], BF16, tag="traw")
                        ttw = t_Tw[:].tensor
                        for (y0, ln, S0) in segs[(b, q)]:
                            eng = dma_eng[ndma[0] % 2]
                            ndma[0] += 1
                            eng.dma_start(
                                out=AP(ttw, (y0 - 32 * q) * 96,
                                       [[96 * 32, us], [96, ln], [1, 96]]),
                                in_=AP(c_base, S0 * CJ + b * NSLOT * ZC,
                                       [[CJ, us], [CJ, ln], [1, 96]]))
                        # deinterleave pairs (SBUF->SBUF DMA):
                        #   T[u, pair*1536 + y*48 + d48] (contiguous blocks)
                        t_T = wpool.tile([128, 96 * 32], BF16, tag="tshear")
                        tt = t_T[:].tensor
                        for pair in range(2 if b != 0 else 1):
                            eng = deint_eng[ndeint[0] % 2]
                            ndeint[0] += 1
                            eng.dma_start(
                                out=AP(tt, pair * 1536,
                                       [[96 * 32, us], [1, 1536]]),
                                in_=AP(ttw, pair * 48,
                                       [[96 * 32, us], [96, 32], [1, 48]]))
                        # T_r = T * r[y]  (bf16, shared by both sets)
                        if not rzero[b]:
                            t_Tr = wpool.tile([128, 96 * 32], BF16, tag="tr",
                                              bufs=2)
                            ttr = t_Tr[:].tensor
                            nc.vector.tensor_mul(
                                AP(ttr, 0, [[3072, us], [1536, 2],
                                            [48, 32], [1, 48]]),
                                AP(tt, 0, [[3072, us], [1536, 2],
                                           [48, 32], [1, 48]]),
                                AP(t_R[:].tensor, b * 128 + 32 * q,
                                   [[NBASE * 128, us], [0, 2],
                                    [1, 32], [0, 48]]))
                        b_sets = [(b, False)] + ([(b, True)] if b != 0 else [])

                        def rhs_ap(src, ch, pair):
                            return AP(src, pair * 1536 + ch * 512,
                                      [[96 * 32, us], [1, 512]])

                        # stream1: L0 @ T -> acc
                        for (bb, xm) in b_sets:
                            si = set_idx[(bb, xm)]
                            pair = 1 if xm else 0
                            for ch in range(3):
                                mm_done[ch] += 1
                                nc.tensor.matmul(
                                    acc[:, 512 * ch:512 * (ch + 1)],
                                    L_ap(si, 0, us), rhs_ap(tt, ch, pair),
                                    start=first_acc[ch],
                                    stop=(mm_done[ch] == n_accmm))
                                first_acc[ch] = False
                        # stream2: D1 @ T_r -> acc
                        if not rzero[b]:
                            for (bb, xm) in b_sets:
                                si = set_idx[(bb, xm)]
                                pair = 1 if xm else 0
                                for ch in range(3):
                                    mm_done[ch] += 1
                                    nc.tensor.matmul(
                                        acc[:, 512 * ch:512 * (ch + 1)],
                                        L_ap(si, 1, us),
                                        rhs_ap(ttr, ch, pair),
                                        start=first_acc[ch],
                                        stop=(mm_done[ch] == n_accmm))
                                    first_acc[ch] = False
                        # stream3: D2 @ T -> pd ; m = wB*pd ; aw += m
                        for (bb, xm) in b_sets:
                            si = set_idx[(bb, xm)]
                            if skip3[si]:
                                continue
                            pair = 1 if xm else 0
                            pd = ppool.tile([128, 1536], F32, tag="pd")
                            for ch in range(3):
                                nc.tensor.matmul(
                                    pd[:, 512 * ch:512 * (ch + 1)],
                                    L_ap(si, 2, us), rhs_ap(tt, ch, pair),
                                    start=True, stop=True)
                            pc = wpool.tile([128, 1536], BF16, tag="pc")
                            nc.scalar.copy(pc[:], pd[:])
                            m = wpool.tile([128, 1536], BF16, tag="mbuf")
                            nc.vector.tensor_mul(
                                AP(m[:].tensor, 0,
                                   [[1536, 128], [48, 32], [1, 48]]),
                                AP(pc[:].tensor, 0,
                                   [[1536, 128], [48, 32], [1, 48]]),
                                W_ap(si, q))
                            nc.gpsimd.tensor_add(aw[:], aw[:], m[:])
                    # evac quarter (flat: acc layout == out_t layout)
                    aw32 = opool.tile([128, 1536], F32, tag="aw32")
                    nc.scalar.copy(aw32[:], aw[:])
                    nc.vector.tensor_add(
                        AP(out_t[:].tensor, 32 * q * 48,
                           [[128 * 48, 128], [1, 1536]]),
                        acc[:], aw32[:])

                # ---- unwind: out = (A + B^T) * inv_norm ----
                for zc2 in range(2):
                    bt = ppool.tile([128, 1536], F32, tag="pd")
                    for zl in range(12):
                        z = zc2 * 12 + zl
                        nc.tensor.transpose(
                            bt[:, 128 * zl:128 * (zl + 1)],
                            AP(out_t[:].tensor, 24 + z,
                               [[128 * 48, 128], [48, 128]]),
                            t_I[:])
                    t_fin = opool.tile([128, 128 * 12], F32, tag="fin")
                    nc.vector.tensor_add(
                        t_fin[:],
                        AP(out_t[:].tensor, zc2 * 12,
                           [[128 * 48, 128], [48, 128], [1, 12]]),
                        AP(bt[:].tensor, bt[:].offset,
                           [[1536, 128], [1, 128], [128, 12]]))
                    nc.vector.tensor_scalar_mul(t_fin[:], t_fin[:], INV_NORM)
                    nc.sync.dma_start(
                        out=AP(d_out[:].tensor, zc2 * 12,
                               [[128 * 24, 128], [24, 128], [1, 12]]),
                        in_=t_fin[:])

            if repeat == 1:
                body()
            else:
                with tc.For_i(0, repeat, 1):
                    body()

    nc.compile()
    return nc


# ---------------------------------------------------------------- entry

_CACHE = {}


def _get(repeat=1):
    key = ("k", repeat)
    if key not in _CACHE:
        tabs = host_prep()
        nc = build_nc(tabs, repeat=repeat)
        _CACHE[key] = (tabs, nc)
    return _CACHE[key]


def make_in_maps(tabs, image):
    import ml_dtypes
    l_bf16 = np.ascontiguousarray(
        tabs["L"].reshape(-1, 128, 128)).astype(ml_dtypes.bfloat16)
    w_bf16 = np.ascontiguousarray(tabs["W2"]).astype(ml_dtypes.bfloat16)
    r_rep = np.broadcast_to(
        tabs["Rtab"].reshape(1, -1), (128, NBASE * 128))
    r_bf16 = np.ascontiguousarray(r_rep).astype(ml_dtypes.bfloat16)
    in_maps = []
    for c in range(N_CORES):
        m = host_inputs(image, c)
        m["l_tab"] = l_bf16
        m["w_tab"] = w_bf16
        m["r_tab"] = r_bf16
        m["ident"] = tabs["ident"]
        in_maps.append(m)
    return in_maps


def run_built(tabs, nc, image):
    in_maps = make_in_maps(tabs, image)
    res = run_bass_kernel_spmd(nc, in_maps, list(range(N_CORES)), trace=False)
    outs = []
    for c in range(N_CORES):
        o = res.results[c]["out"]                 # [x, y, ZC]
        outs.append(np.transpose(o, (1, 0, 2)))   # [y, x, ZC]
    full = np.concatenate(outs, axis=2)           # [128, 128, 192]
    return full[None].astype(np.float32)


def kernel(image):
    image = np.asarray(image, dtype=np.float32)
    tabs, nc = _get(repeat=1)
    return run_built(tabs, nc, image)


# revision 24
# speedup vs baseline: 1.0013x; 1.0013x over previous
"""Back-projection (nn_BackProjectionNet) Trainium2 Bass kernel.

Full inputs in, full outputs out. Sharding: z (last dim, 192) split over 8
cores, 24 z-planes each; no collectives (each core fully independent).

Math: out[y,x,z] = sum_n lerp_x(padded_slice_n; sx_n(y,x)) / (120 + 1e-11)
with sx = sp*yc + cp*xc + C. The 120 angles fold 8-ways onto 16 base angles
in [0,45deg] ("sin convention"), 31 (base, xmirror) sets.

Per set, with K[x] = floor(cpx*xc)-KMIN, S[y] = floor(spa*yc+C)+KMIN,
k = frac_x, r = frac_y, wB = max(k+r-1, 0), G_j = C[S[y]+K[x]+j]:

  out_set = G0 + k(G1-G0) + r(G1-G0) + wB(G0-2G1+G2)        (exact)

Device pipeline per (base, y-quarter):
  1. affine-segment shear DMA (one 4D instruction per maximal slope-1
     segment of S - ~5/quarter vs ~26 constant-S runs):
       T[u<uspan, pair*1536 + y*48 + d48] = C[S[y]+u, b, pair-slots, z]
     pair-major blocks so every matmul rhs chunk is CONTIGUOUS - the PE
     runs strided rhs ~4x slower (~970ns vs 233ns per 512-col matmul).
  2. PE streams into PSUM acc[x, y*48+d48] (both pairs accumulate into the
     same columns - the A/B member sums the unwind needs anyway):
       S1: L0 @ T      L0 = (1-k) at row K, k at K+1   -> G0 + k(G1-G0)
       S2: D1 @ T_r    D1 = -+1 at K,K+1; T_r = T*r[y] (DVE bf16)
       S3: D2 @ T -> pd (transient PSUM); D2 = +1,-2,+1 at K..K+2
  3. Act: pc = bf16(pd) (Pool can't read PSUM); DVE: m = wB .* pc at
     2x bf16 rate; Pool: aw += m
  4. evac: OUT-quarter = acc + cvt(aw), flat (acc layout == out_t layout)
Unwind: frame B is PE-transposed and added, scale by 1/(120+1e-11).
Host does only layout (pad, z-shard, slice flips, final transpose).
"""

import math
import numpy as np

import concourse.bacc as bacc
import concourse.mybir as mybir
from concourse import tile
from concourse.ap import AP
from concourse.bass_utils import run_bass_kernel_spmd

NA, LR, LZ, PAD = 120, 128, 192, 27
LP = LR + 2 * PAD          # 182
CEN = (LP - 1) / 2.0       # 90.5
N_CORES = 8
ZC = LZ // N_CORES         # 24
CROWS = 288                # padded combined-slice rows
NBASE = 16
NSLOT = 4                  # job slots per base in C buffer
DT = mybir.dt
INV_NORM = float(np.float32(1.0 / (120.0 + 1e-11)))

F32, BF16 = DT.float32, DT.bfloat16
CJ = NBASE * NSLOT * ZC    # C row stride in elements = 1536


# ---------------------------------------------------------------- host math

def _job_slots():
    """Per base: slot -> (plain_member, flipped_member) or None.
    Slot order: [plain-A, plain-B, xm-A, xm-B]."""
    slots = {}
    for b in range(NBASE):
        if b == 0:
            slots[b] = [(60, 0), (90, 30), None, None]
        elif b == 15:
            slots[b] = [(75, 15), None, None, (105, 45)]
        else:
            slots[b] = [((60 + b) % 120, b), (90 - b, 30 - b),
                        (60 - b, 120 - b), (90 + b, 30 + b)]
    return slots


def _base_tables(b, xm):
    a = 2 * math.pi * b / NA
    cpa, spa = math.sin(a), math.cos(a)
    cpx = -cpa if xm else cpa
    yc = np.arange(PAD, PAD + LR, dtype=np.float64) - CEN
    xc = np.arange(PAD, PAD + LR, dtype=np.float64) - CEN
    ay = spa * yc + CEN
    bx = cpx * xc
    Sf, Kf = np.floor(ay), np.floor(bx)
    KMIN = int(Kf.min())
    K = (Kf - KMIN).astype(np.int64)        # [x] >= 0
    S = (Sf + KMIN).astype(np.int64)        # [y]
    r = (ay - Sf).astype(np.float64)        # frac_y [y]
    k = (bx - Kf).astype(np.float64)        # frac_x [x]
    wB = np.maximum(r[None, :] + k[:, None] - 1.0, 0.0)   # [x, y]
    return S, K, r, k, wB


def host_prep():
    """Build all constant tables + plans."""
    slots = _job_slots()
    sets = []            # (b, xm)
    for b in range(NBASE):
        sets.append((b, False))
        if b != 0:
            sets.append((b, True))
    nset = len(sets)     # 31
    L = np.zeros((nset, 3, 128, 128), np.float32)   # [set, stream, u, x]
    W2 = np.zeros((nset, 128, 128), np.float32)     # [set, x, y] = wB
    Rtab = np.zeros((NBASE, 128), np.float32)       # [base, y] = r
    skip3 = {}
    Sbase = {}
    xs = np.arange(128)
    for i, (b, xm) in enumerate(sets):
        S, K, r, k, wB = _base_tables(b, xm)
        if b in Sbase:
            assert np.array_equal(Sbase[b][0], S)
            assert np.allclose(Sbase[b][1], r)
        Sbase[b] = (S, r)
        Rtab[b] = r.astype(np.float32)
        L[i, 0][K, xs] = 1.0 - k
        L[i, 0][K + 1, xs] = k
        L[i, 1][K, xs] = -1.0
        L[i, 1][K + 1, xs] = 1.0
        L[i, 2][K, xs] = 1.0
        L[i, 2][K + 1, xs] = -2.0
        L[i, 2][K + 2, xs] = 1.0
        W2[i] = wB
        skip3[i] = bool(np.all(wB == 0.0))
    rzero = {b: bool(np.all(Rtab[b] == 0.0)) for b in range(NBASE)}
    # per-base tap-row span (rows with any nonzero L entry, over live streams)
    uspan = {}
    segs = {}
    for b in range(NBASE):
        hi = 0
        for i, (bb, xm) in enumerate(sets):
            if bb != b:
                continue
            streams = [0] + ([] if rzero[b] else [1]) \
                + ([] if skip3[i] else [2])
            for j in streams:
                rows = np.nonzero(np.abs(L[i, j]).max(axis=1) > 0)[0]
                hi = max(hi, int(rows.max()) + 1)
        uspan[b] = hi
        # affine (slope-1) segments of S within each y-quarter
        S = Sbase[b][0]
        for q in range(4):
            rr = []
            y = 32 * q
            while y < 32 * (q + 1):
                y1 = y
                while y1 + 1 < 32 * (q + 1) and S[y1 + 1] == S[y1] + 1:
                    y1 += 1
                rr.append((y, y1 - y + 1, int(S[y])))
                assert S[y] >= 0 and S[y] + (y1 - y) + hi <= CROWS, (b, q, y)
                y = y1 + 1
            segs[(b, q)] = rr
    ident = np.eye(128, dtype=np.float32)
    return dict(slots=slots, sets=sets, L=L, W2=W2, Rtab=Rtab, ident=ident,
                segs=segs, uspan=uspan, skip3=skip3, rzero=rzero)


def host_inputs(image, core):
    """Per-core input arrays. image [1,120,128,192] f32."""
    z0 = core * ZC
    img = np.asarray(image)[0, :, :, z0:z0 + ZC]               # [120,128,ZC]
    img_p = np.pad(img, ((0, 0), (PAD, PAD), (0, 0)))          # [120,182,ZC]
    slots = _job_slots()
    sp = np.zeros((NBASE * NSLOT, LP, ZC), np.float32)
    sf = np.zeros((NBASE * NSLOT, LP, ZC), np.float32)
    for b in range(NBASE):
        for s in range(NSLOT):
            j = slots[b][s]
            if j is None:
                continue
            mp, mf = j
            sp[b * NSLOT + s] = img_p[mp]
            sf[b * NSLOT + s] = img_p[mf][::-1]
    return {"slices_p": sp, "slices_f": sf}


# ---------------------------------------------------------------- device

def build_nc(tabs, repeat=1, nbases=NBASE, nquarters=4):
    sets, segs, skip3, rzero = tabs["sets"], tabs["segs"], tabs["skip3"], \
        tabs["rzero"]
    uspan = tabs["uspan"]
    nset = len(sets)
    set_idx = {bs: i for i, bs in enumerate(sets)}

    nc = bacc.Bacc("TRN2", target_bir_lowering=False, debug=False,
                   num_devices=N_CORES)
    d_sp = nc.dram_tensor("slices_p", [NBASE * NSLOT, LP, ZC], F32,
                          kind="ExternalInput")
    d_sf = nc.dram_tensor("slices_f", [NBASE * NSLOT, LP, ZC], F32,
                          kind="ExternalInput")
    d_L = nc.dram_tensor("l_tab", [nset * 3, 128, 128], BF16,
                         kind="ExternalInput")
    d_W = nc.dram_tensor("w_tab", [nset, 128, 128], BF16,
                         kind="ExternalInput")
    d_R = nc.dram_tensor("r_tab", [128, NBASE * 128], BF16,
                         kind="ExternalInput")
    d_I = nc.dram_tensor("ident", [128, 128], F32, kind="ExternalInput")
    d_out = nc.dram_tensor("out", [128, 128, ZC], F32, kind="ExternalOutput")

    with tile.TileContext(nc) as tc:
        with tc.tile_pool(name="const", bufs=1) as cpool, \
             tc.tile_pool(name="work", bufs=3) as wpool, \
             tc.tile_pool(name="once", bufs=1) as opool, \
             tc.tile_pool(name="accs", bufs=1) as apool, \
             tc.tile_pool(name="dram", bufs=1, space="DRAM") as dpool, \
             tc.tile_pool(name="psum", bufs=1, space="PSUM") as ppool:

            d_C = dpool.tile([CROWS * NBASE * NSLOT * ZC], BF16, tag="cbuf")
            c_base = d_C[:].tensor

            # ---- constants to SBUF (outside timing loop) ----
            t_L = cpool.tile([128, nset * 3 * 128], BF16, tag="ltab")
            nc.sync.dma_start(
                out=t_L[:],
                in_=AP(d_L[:].tensor, 0,
                       [[128, 128], [128 * 128, nset * 3], [1, 128]]))
            t_W = cpool.tile([128, nset * 128], BF16, tag="wtab")
            nc.sync.dma_start(
                out=t_W[:],
                in_=AP(d_W[:].tensor, 0,
                       [[128, 128], [128 * 128, nset], [1, 128]]))
            t_R = cpool.tile([128, NBASE * 128], BF16, tag="rtab")
            nc.sync.dma_start(out=t_R[:], in_=d_R[:])
            t_I = cpool.tile([128, 128], F32, tag="ident")
            nc.sync.dma_start(out=t_I[:], in_=d_I[:])

            def L_ap(si, j, us):   # lhsT [us, 128] bf16
                return AP(t_L[:].tensor, (si * 3 + j) * 128,
                          [[nset * 3 * 128, us], [1, 128]])

            def W_ap(si, q):       # [128, (y32), (d48 bcast)] bf16
                return AP(t_W[:].tensor, si * 128 + 32 * q,
                          [[nset * 128, 128], [1, 32], [0, 48]])

            dma_eng = [nc.sync, nc.scalar]

            # ---- zero C buffer once (combine only writes rows < 182) ----
            t_z = opool.tile([128, 3456], BF16, tag="zero")
            nc.gpsimd.memset(t_z[:], 0)
            nc.sync.dma_start(
                out=AP(c_base, 0, [[3456, 128], [1, 3456]]),
                in_=t_z[:])

            def body():
                # ---- combine slices: C = P + flip(F) (host pre-flipped) ----
                t_p = opool.tile([64, LP * ZC], F32, tag="slp")
                t_f = opool.tile([64, LP * ZC], F32, tag="slf")
                nc.sync.dma_start(
                    out=t_p[:],
                    in_=AP(d_sp[:].tensor, 0, [[LP * ZC, 64], [1, LP * ZC]]))
                nc.scalar.dma_start(
                    out=t_f[:],
                    in_=AP(d_sf[:].tensor, 0, [[LP * ZC, 64], [1, LP * ZC]]))
                t_c = opool.tile([64, LP * ZC], BF16, tag="slc")
                nc.vector.tensor_add(t_c[:], t_p[:], t_f[:])
                nc.sync.dma_start(
                    out=AP(c_base, 0, [[ZC, 64], [CJ, LP], [1, ZC]]),
                    in_=t_c[:])

                # ---- main loop ----
                out_t = apool.tile([128, 128 * 48], F32, tag="outbuf")
                ndma = [0]
                ndeint = [0]
                deint_eng = [nc.sync, nc.scalar]
                for q in range(nquarters):
                    acc = ppool.tile([128, 1536], F32, tag="acc")
                    aw = apool.tile([128, 1536], BF16, tag="aw")
                    nc.gpsimd.memset(aw[:], 0)
                    # per-ch acc matmul counts for start/stop flags
                    n_accmm = 0
                    for b in range(nbases):
                        n_accmm += (2 if b != 0 else 1)     # plain,xm
                        if not rzero[b]:
                            n_accmm += (2 if b != 0 else 1)
                    mm_done = [0] * 3
                    first_acc = [True] * 3
                    s3q = []

                    def emit_s3(job):
                        si, pair, tt, us, qq = job
                        pd = ppool.tile([128, 1536], F32, tag="pd")
                        for ch in range(3):
                            nc.tensor.matmul(
                                pd[:, 512 * ch:512 * (ch + 1)],
                                L_ap(si, 2, us),
                                AP(tt, pair * 1536 + ch * 512,
                                   [[96 * 32, us], [1, 512]]),
                                start=True, stop=True)
                        pc = wpool.tile([128, 1536], BF16, tag="pc")
                        nc.scalar.copy(pc[:], pd[:])
                        m = wpool.tile([128, 1536], BF16, tag="mbuf")
                        nc.vector.tensor_mul(
                            AP(m[:].tensor, 0,
                               [[1536, 128], [48, 32], [1, 48]]),
                            AP(pc[:].tensor, 0,
                               [[1536, 128], [48, 32], [1, 48]]),
                            W_ap(si, qq))
                        nc.gpsimd.tensor_add(aw[:], aw[:], m[:])

                    for b in range(nbases):
                        us = uspan[b]
                        # affine-segment shear DMA -> Traw[u<us, y*96 + d96]
                        t_Tw = wpool.tile([128, 96 * 32], BF16, tag="traw")
                        ttw = t_Tw[:].tensor
                        for (y0, ln, S0) in segs[(b, q)]:
                            eng = dma_eng[ndma[0] % 2]
                            ndma[0] += 1
                            eng.dma_start(
                                out=AP(ttw, (y0 - 32 * q) * 96,
                                       [[96 * 32, us], [96, ln], [1, 96]]),
                                in_=AP(c_base, S0 * CJ + b * NSLOT * ZC,
                                       [[CJ, us], [CJ, ln], [1, 96]]))
                        # deinterleave pairs (SBUF->SBUF DMA):
                        #   T[u, pair*1536 + y*48 + d48] (contiguous blocks)
                        t_T = wpool.tile([128, 96 * 32], BF16, tag="tshear")
                        tt = t_T[:].tensor
                        for pair in range(2 if b != 0 else 1):
                            eng = deint_eng[ndeint[0] % 2]
                            ndeint[0] += 1
                            eng.dma_start(
                                out=AP(tt, pair * 1536,
                                       [[96 * 32, us], [1, 1536]]),
                                in_=AP(ttw, pair * 48,
                                       [[96 * 32, us], [96, 32], [1, 48]]))
                        # T_r = T * r[y]  (bf16, shared by both sets)
                        if not rzero[b]:
                            t_Tr = wpool.tile([128, 96 * 32], BF16, tag="tr",
                                              bufs=2)
                            ttr = t_Tr[:].tensor
                            nc.vector.tensor_mul(
                                AP(ttr, 0, [[3072, us], [1536, 2],
                                            [48, 32], [1, 48]]),
                                AP(tt, 0, [[3072, us], [1536, 2],
                                           [48, 32], [1, 48]]),
                                AP(t_R[:].tensor, b * 128 + 32 * q,
                                   [[NBASE * 128, us], [0, 2],
                                    [1, 32], [0, 48]]))
                        b_sets = [(b, False)] + ([(b, True)] if b != 0 else [])

                        def rhs_ap(src, ch, pair):
                            return AP(src, pair * 1536 + ch * 512,
                                      [[96 * 32, us], [1, 512]])

                        # stream1: L0 @ T -> acc
                        for (bb, xm) in b_sets:
                            si = set_idx[(bb, xm)]
                            pair = 1 if xm else 0
                            for ch in range(3):
                                mm_done[ch] += 1
                                nc.tensor.matmul(
                                    acc[:, 512 * ch:512 * (ch + 1)],
                                    L_ap(si, 0, us), rhs_ap(tt, ch, pair),
                                    start=first_acc[ch],
                                    stop=(mm_done[ch] == n_accmm))
                                first_acc[ch] = False
                        # one pending stream3 job between acc streams:
                        # PE never has back-to-back pd groups, so Act's pd
                        # drain hides under the next acc-stream matmuls
                        if s3q:
                            emit_s3(s3q.pop(0))
                        # stream2: D1 @ T_r -> acc
                        if not rzero[b]:
                            for (bb, xm) in b_sets:
                                si = set_idx[(bb, xm)]
                                pair = 1 if xm else 0
                                for ch in range(3):
                                    mm_done[ch] += 1
                                    nc.tensor.matmul(
                                        acc[:, 512 * ch:512 * (ch + 1)],
                                        L_ap(si, 1, us),
                                        rhs_ap(ttr, ch, pair),
                                        start=first_acc[ch],
                                        stop=(mm_done[ch] == n_accmm))
                                    first_acc[ch] = False
                        if s3q:
                            emit_s3(s3q.pop(0))
                        # stream3: D2 @ T -> pd ; m = wB*pd ; aw += m
                        for (bb, xm) in b_sets:
                            si = set_idx[(bb, xm)]
                            if skip3[si]:
                                continue
                            pair = 1 if xm else 0
                            s3q.append((si, 1 if xm else 0, tt, us, q))
                    while s3q:
                        emit_s3(s3q.pop(0))
                    # evac quarter (flat: acc layout == out_t layout)
                    aw32 = opool.tile([128, 1536], F32, tag="aw32")
                    nc.scalar.copy(aw32[:], aw[:])
                    nc.vector.tensor_add(
                        AP(out_t[:].tensor, 32 * q * 48,
                           [[128 * 48, 128], [1, 1536]]),
                        acc[:], aw32[:])

                # ---- unwind: out = (A + B^T) * inv_norm ----
                for zc2 in range(2):
                    bt = ppool.tile([128, 1536], F32, tag="pd")
                    for zl in range(12):
                        z = zc2 * 12 + zl
                        nc.tensor.transpose(
                            bt[:, 128 * zl:128 * (zl + 1)],
                            AP(out_t[:].tensor, 24 + z,
                               [[128 * 48, 128], [48, 128]]),
                            t_I[:])
                    t_fin = opool.tile([128, 128 * 12], F32, tag="fin")
                    nc.vector.tensor_add(
                        t_fin[:],
                        AP(out_t[:].tensor, zc2 * 12,
                           [[128 * 48, 128], [48, 128], [1, 12]]),
                        AP(bt[:].tensor, bt[:].offset,
                           [[1536, 128], [1, 128], [128, 12]]))
                    nc.vector.tensor_scalar_mul(t_fin[:], t_fin[:], INV_NORM)
                    nc.sync.dma_start(
                        out=AP(d_out[:].tensor, zc2 * 12,
                               [[128 * 24, 128], [24, 128], [1, 12]]),
                        in_=t_fin[:])

            if repeat == 1:
                body()
            else:
                with tc.For_i(0, repeat, 1):
                    body()

    nc.compile()
    return nc


# ---------------------------------------------------------------- entry

_CACHE = {}


def _get(repeat=1):
    key = ("k", repeat)
    if key not in _CACHE:
        tabs = host_prep()
        nc = build_nc(tabs, repeat=repeat)
        _CACHE[key] = (tabs, nc)
    return _CACHE[key]


def make_in_maps(tabs, image):
    import ml_dtypes
    l_bf16 = np.ascontiguousarray(
        tabs["L"].reshape(-1, 128, 128)).astype(ml_dtypes.bfloat16)
    w_bf16 = np.ascontiguousarray(tabs["W2"]).astype(ml_dtypes.bfloat16)
    r_rep = np.broadcast_to(
        tabs["Rtab"].reshape(1, -1), (128, NBASE * 128))
    r_bf16 = np.ascontiguousarray(r_rep).astype(ml_dtypes.bfloat16)
    in_maps = []
    for c in range(N_CORES):
        m = host_inputs(image, c)
        m["l_tab"] = l_bf16
        m["w_tab"] = w_bf16
        m["r_tab"] = r_bf16
        m["ident"] = tabs["ident"]
        in_maps.append(m)
    return in_maps


def run_built(tabs, nc, image):
    in_maps = make_in_maps(tabs, image)
    res = run_bass_kernel_spmd(nc, in_maps, list(range(N_CORES)), trace=False)
    outs = []
    for c in range(N_CORES):
        o = res.results[c]["out"]                 # [x, y, ZC]
        outs.append(np.transpose(o, (1, 0, 2)))   # [y, x, ZC]
    full = np.concatenate(outs, axis=2)           # [128, 128, 192]
    return full[None].astype(np.float32)


def kernel(image):
    image = np.asarray(image, dtype=np.float32)
    tabs, nc = _get(repeat=1)
    return run_built(tabs, nc, image)
